# revision 4
# baseline (speedup 1.0000x reference)
"""Bahdanau-attention kernel (nn_Attention_61357902791110) for Trainium2.

Data-parallel over 8 NeuronCores: the batch (128) is sharded 16 per core;
all weights are replicated. Inputs are cast/transposed host-side into the
layouts the device kernel wants; the Bass/Tile program per core computes

  att1 = enc @ We + be; att2 = dec @ Wd + bd
  s     = relu(att1 + att2[:,None,:]) @ Wf        (+bf dropped: softmax-inv.)
  alpha = softmax(s, axis=P)
  awe   = einsum('pe,p->e', enc, alpha)  per batch row

and returns (awe [128,2048] f32, alpha [128,196] f32).

Self-contained: only needs the toolchain at /opt/trn_rl_repo and 8 visible
neuron cores via jax.
"""

import sys

sys.path.insert(0, "/opt/trn_rl_repo")

from contextlib import ExitStack
from dataclasses import dataclass

import numpy as np

import concourse.bacc as bacc
import concourse.bass as bass  # noqa: F401  (engine types referenced via nc)
import concourse.tile as tile
from concourse import mybir

FP32 = mybir.dt.float32
BF16 = mybir.dt.bfloat16
AXX = mybir.AxisListType.X
AF = mybir.ActivationFunctionType
ALU = mybir.AluOpType

N_CORES = 8


@dataclass
class Cfg:
    BL: int = 16      # batches per core
    P: int = 196      # pixels
    E: int = 2048     # encoder dim
    A: int = 512      # attention dim
    D: int = 512      # decoder dim
    reps: int = 1     # in-kernel repetitions (timing harnesses only)

    @property
    def KO(self): return self.E // 128
    @property
    def AM(self): return self.A // 128
    @property
    def KD(self): return self.D // 128
    @property
    def NT(self): return 2 * self.P      # tokens per b-pair
    @property
    def NJ(self): return self.BL // 2
    @property
    def TL(self): return self.BL * self.P
    @property
    def NE(self): return self.E // 512   # awe psum chunks
    @property
    def P1(self): return min(self.P, 128)
    @property
    def P2(self): return self.P - self.P1


def emit_kernel(ctx: ExitStack, tc: tile.TileContext, ins: dict, outs: dict, cfg: Cfg):
    nc = tc.nc
    c = cfg
    encT_D, encN_D = ins["encT"], ins["encN"]
    we_D, wd_D, dect_D = ins["we"], ins["wd"], ins["dect"]
    bea_D, wf_D, id_D = ins["bea"], ins["wf"], ins["ident"]
    awe_D, alpha_D = outs["awe"], outs["alpha"]

    const = ctx.enter_context(tc.tile_pool(name="const", bufs=1))
    encT_pool = ctx.enter_context(tc.tile_pool(name="encT", bufs=2))
    encN_pool = ctx.enter_context(tc.tile_pool(name="encN", bufs=16))
    relu_pool = ctx.enter_context(tc.tile_pool(name="relu", bufs=2))
    small = ctx.enter_context(tc.tile_pool(name="small", bufs=3))
    pm = ctx.enter_context(tc.tile_pool(name="pm", bufs=1, space="PSUM"))
    pswf = ctx.enter_context(tc.tile_pool(name="pswf", bufs=1, space="PSUM"))
    pstr = ctx.enter_context(tc.tile_pool(name="pstr", bufs=1, space="PSUM"))
    psawe = ctx.enter_context(tc.tile_pool(name="psawe", bufs=2, space="PSUM"))

    NCH = 4
    KG = c.KO // NCH

    we_sb = const.tile([128, c.KO * c.A], BF16, tag="we")
    we_v = we_sb.rearrange("p (k a) -> p k a", k=c.KO)
    for g in range(NCH):
        nc.sync.dma_start(we_v[:, g * KG : (g + 1) * KG, :],
                          we_D[:, g * KG : (g + 1) * KG, :])

    def emit_dma_encT(j):
        t0 = j * c.NT
        encT = encT_pool.tile([128, c.KO * c.NT], BF16, tag="encT")
        encT_v = encT.rearrange("p (k t) -> p k t", k=c.KO)
        for g in range(NCH):
            nc.sync.dma_start(encT_v[:, g * KG : (g + 1) * KG, :],
                              encT_D[:, g * KG : (g + 1) * KG, t0 : t0 + c.NT])
        return encT_v

    SEG = c.P // 2          # K-segment for the awe matmuls
    NSEG = 8                # segments per quad (4 batches x 2)

    def emit_dma_encN(q):
        # 8 tiles of [SEG, E] covering the quad's 4*P tokens
        t0 = q * 4 * c.P
        encN_tiles = []
        for s in range(NSEG):
            ta = encN_pool.tile([SEG, c.E], BF16, tag="encN")
            nc.sync.dma_start(ta[:], encN_D[t0 + s * SEG : t0 + (s + 1) * SEG, :])
            encN_tiles.append(ta)
        return encN_tiles

    def emit_main_mm(j, encT_v):
        ps_main = [
            pm.tile([128, c.NT], FP32, tag=f"m{am}", name=f"psm{am}")
            for am in range(c.AM)
        ]
        for ko in range(c.KO):
            for am in range(c.AM):
                nc.tensor.matmul(
                    ps_main[am][:],
                    lhsT=we_sb[:, ko * c.A + am * 128 : ko * c.A + (am + 1) * 128],
                    rhs=encT_v[:, ko, :],
                    start=(ko == 0), stop=(ko == c.KO - 1),
                )
        return ps_main

    def emit_phase0():
        wd_sb = const.tile([128, c.KD * c.A], BF16, tag="wd")
        nc.sync.dma_start(wd_sb.rearrange("p (k a) -> p k a", k=c.KD), wd_D[:])
        dect_sb = const.tile([128, c.KD * c.BL], BF16, tag="dect")
        nc.sync.dma_start(dect_sb.rearrange("p (k b) -> p k b", k=c.KD), dect_D[:])
        bea_sb = const.tile([128, c.AM], FP32, tag="bea")
        nc.sync.dma_start(bea_sb[:], bea_D[:])
        wf_sb = const.tile([128, c.AM], BF16, tag="wf")
        nc.sync.dma_start(wf_sb[:], wf_D[:])
        id_sb = const.tile([128, 128], BF16, tag="ident")
        nc.sync.dma_start(id_sb[:], id_D[:])
        cT_sb = const.tile([128, c.AM * c.BL], FP32, tag="cT")
        for am in range(c.AM):
            ps = psawe.tile([128, c.BL], FP32, tag="a")
            for kd in range(c.KD):
                nc.tensor.matmul(
                    ps[:],
                    lhsT=wd_sb[:, kd * c.A + am * 128 : kd * c.A + (am + 1) * 128],
                    rhs=dect_sb[:, kd * c.BL : (kd + 1) * c.BL],
                    start=(kd == 0), stop=(kd == c.KD - 1),
                )
            nc.vector.tensor_scalar_add(
                cT_sb[:, am * c.BL : (am + 1) * c.BL], ps[:],
                bea_sb[:, am : am + 1],
            )
        return wf_sb, id_sb, cT_sb

    def emit_score_pre(j, ps_main, wf_sb, cT_sb, sc_q):
        """relu + Wf reduction + move scores into quad tile rows."""
        relu = relu_pool.tile([128, c.AM * c.NT], BF16, tag="relu")
        for am in range(c.AM):
            for h in range(2):
                b = 2 * j + h
                nc.vector.tensor_scalar(
                    relu[:, am * c.NT + h * c.P : am * c.NT + (h + 1) * c.P],
                    ps_main[am][:, h * c.P : (h + 1) * c.P],
                    cT_sb[:, am * c.BL + b : am * c.BL + b + 1],
                    0.0,
                    op0=ALU.add, op1=ALU.max,
                )
        ps_s = pswf.tile([1, c.NT], FP32, tag="s")
        for am in range(c.AM):
            nc.tensor.matmul(
                ps_s[:],
                lhsT=wf_sb[:, am : am + 1],
                rhs=relu[:, am * c.NT : (am + 1) * c.NT],
                start=(am == 0), stop=(am == c.AM - 1),
            )
        scr = small.tile([1, c.NT], FP32, tag="scr")
        nc.scalar.copy(scr[:], ps_s[:])
        r0 = 2 * (j % 2)
        for h in range(2):
            nc.sync.dma_start(sc_q[r0 + h : r0 + h + 1, :],
                              scr[0:1, h * c.P : (h + 1) * c.P])

    def emit_softmax_q(q, sc_q, id_sb):
        """Quad softmax [4, P], alpha out, and the block-diagonal alphaT
        [SEG, NSEG*4] used as stationary operands by the awe matmuls."""
        nmax = small.tile([4, 1], FP32, tag="nmax")
        nc.vector.reduce_max(nmax[:], sc_q[:], axis=AXX, negate=True)
        esb = small.tile([4, c.P], FP32, tag="esb")
        ssum = small.tile([4, 1], FP32, tag="ssum")
        nc.scalar.activation(esb[:], sc_q[:], AF.Exp, bias=nmax[:], accum_out=ssum[:])
        rsum = small.tile([4, 1], FP32, tag="rsum")
        nc.vector.reciprocal(rsum[:], ssum[:])
        al_f = small.tile([4, c.P], FP32, tag="al_f")
        nc.vector.tensor_scalar_mul(al_f[:], esb[:], rsum[:])
        al_b = small.tile([4, c.P], BF16, tag="al_b")
        nc.vector.tensor_copy(al_b[:], al_f[:])
        nc.sync.dma_start(alpha_D[4 * q : 4 * q + 4, :], al_f[:])
        pt = pstr.tile([SEG, 8], BF16, tag="t")
        nc.tensor.transpose(pt[:, 0:4], al_b[:, 0:SEG], id_sb[0:4, 0:4])
        nc.tensor.transpose(pt[:, 4:8], al_b[:, SEG : c.P], id_sb[0:4, 0:4])
        alphaT = small.tile([SEG, NSEG * 4], BF16, tag="alphaT")
        nc.gpsimd.memset(alphaT[:], 0.0)
        for s in range(NSEG):
            b_idx = s // 2
            half = s % 2
            nc.vector.tensor_copy(
                alphaT[:, s * 4 + b_idx : s * 4 + b_idx + 1],
                pt[:, half * 4 + b_idx : half * 4 + b_idx + 1],
            )
        return alphaT

    def emit_awe_q(q, alphaT, encN_tiles):
        awe_sb = small.tile([4, c.E], FP32, tag="awe_sb")
        for n in range(c.NE):
            pa = psawe.tile([4, 512], FP32, tag="a")
            for s in range(NSEG):
                nc.tensor.matmul(
                    pa[:],
                    lhsT=alphaT[:, s * 4 : (s + 1) * 4],
                    rhs=encN_tiles[s][:, n * 512 : (n + 1) * 512],
                    start=(s == 0), stop=(s == NSEG - 1),
                )
            nc.scalar.copy(awe_sb[:, n * 512 : (n + 1) * 512], pa[:])
        nc.sync.dma_start(awe_D[4 * q : 4 * q + 4, :], awe_sb[:])

    def body():
        consts = None
        prev = None   # (q, alphaT, encN_tiles)
        sc_q = None
        pend = None
        for j in range(c.NJ):
            q, jh = divmod(j, 2)
            encT_v = emit_dma_encT(j)
            ps_main = emit_main_mm(j, encT_v)
            if j == 0:
                consts = emit_phase0()
            wf_sb, id_sb, cT_sb = consts
            if jh == 0:
                sc_q = small.tile([4, c.P], FP32, tag="sc", name="sc_q")
                pend = emit_dma_encN(q)
            if jh == 1 and prev is not None:
                emit_awe_q(prev[0], prev[1], prev[2])
            emit_score_pre(j, ps_main, wf_sb, cT_sb, sc_q)
            if jh == 1:
                alphaT = emit_softmax_q(q, sc_q, id_sb)
                prev = (q, alphaT, pend)
        emit_awe_q(prev[0], prev[1], prev[2])

    if c.reps == 1:
        body()
    else:
        with tc.For_i(0, c.reps, 1):
            body()


def build_program(cfg: Cfg):
    c = cfg
    nc = bacc.Bacc("TRN2", target_bir_lowering=False, debug=False)
    ins = {}
    outs = {}
    ins["encT"] = nc.dram_tensor("encT", [128, c.KO, c.TL], BF16, kind="ExternalInput").ap()
    ins["encN"] = nc.dram_tensor("encN", [c.TL, c.E], BF16, kind="ExternalInput").ap()
    ins["we"] = nc.dram_tensor("we", [128, c.KO, c.A], BF16, kind="ExternalInput").ap()
    ins["wd"] = nc.dram_tensor("wd", [128, c.KD, c.A], BF16, kind="ExternalInput").ap()
    ins["dect"] = nc.dram_tensor("dect", [128, c.KD, c.BL], BF16, kind="ExternalInput").ap()
    ins["bea"] = nc.dram_tensor("bea", [128, c.AM], FP32, kind="ExternalInput").ap()
    ins["wf"] = nc.dram_tensor("wf", [128, c.AM], BF16, kind="ExternalInput").ap()
    ins["ident"] = nc.dram_tensor("ident", [128, 128], BF16, kind="ExternalInput").ap()
    outs["awe"] = nc.dram_tensor("awe", [c.BL, c.E], FP32, kind="ExternalOutput").ap()
    outs["alpha"] = nc.dram_tensor("alpha", [c.BL, c.P], FP32, kind="ExternalOutput").ap()

    with tile.TileContext(nc) as tc:
        with ExitStack() as ctx:
            emit_kernel(ctx, tc, ins, outs, cfg)
    nc.compile()
    return nc


def build_runner(nc, n_cores):
    """Persistent SPMD runner (port of bass2jax.run_bass_via_pjrt that keeps
    the jitted executable so repeated calls don't re-trace)."""
    import jax
    import jax.core
    from jax.experimental.shard_map import shard_map
    from jax.sharding import Mesh, PartitionSpec

    from concourse.bass2jax import (
        _bass_exec_p,
        install_neuronx_cc_hook,
        partition_id_tensor,
    )

    install_neuronx_cc_hook()
    partition_name = nc.partition_id_tensor.name if nc.partition_id_tensor else None
    in_names, out_names, out_avals, zero_outs = [], [], [], []
    for alloc in nc.m.functions[0].allocations:
        if not isinstance(alloc, mybir.MemoryLocationSet):
            continue
        name = alloc.memorylocations[0].name
        if alloc.kind == "ExternalInput":
            if name != partition_name:
                in_names.append(name)
        elif alloc.kind == "ExternalOutput":
            out_names.append(name)
            shape = tuple(alloc.tensor_shape)
            dtype = mybir.dt.np(alloc.dtype)
            out_avals.append(jax.core.ShapedArray(shape, dtype))
            zero_outs.append(np.zeros(shape, dtype))
    n_params = len(in_names)
    n_outs = len(out_avals)
    all_in = in_names + out_names + ([partition_name] if partition_name else [])

    def _body(*args):
        operands = list(args)
        if partition_name is not None:
            operands.append(partition_id_tensor())
        outs = _bass_exec_p.bind(
            *operands,
            out_avals=tuple(out_avals),
            in_names=tuple(all_in),
            out_names=tuple(out_names),
            lowering_input_output_aliases=(),
            sim_require_finite=True,
            sim_require_nnan=True,
            nc=nc,
        )
        return tuple(outs)

    devices = jax.devices()[:n_cores]
    mesh = Mesh(np.asarray(devices), ("core",))
    donate = tuple(range(n_params, n_params + n_outs))
    sharded = jax.jit(
        shard_map(
            _body,
            mesh=mesh,
            in_specs=(PartitionSpec("core"),) * (n_params + n_outs),
            out_specs=(PartitionSpec("core"),) * n_outs,
            check_rep=False,
        ),
        donate_argnums=donate,
        keep_unused=True,
    )

    class Runner:
        def __init__(self):
            self.in_names = in_names
            self.out_names = out_names
            self.out_avals = out_avals
            self.mesh = mesh
            self.n_cores = n_cores

        def prep(self, in_maps):
            return [
                np.concatenate([np.asarray(m[name]) for m in in_maps], axis=0)
                for name in in_names
            ]

        def zeros(self):
            return [
                np.zeros((n_cores * z.shape[0], *z.shape[1:]), z.dtype)
                for z in zero_outs
            ]

        def call(self, concat_in, concat_zeros):
            return sharded(*concat_in, *concat_zeros)

        def run(self, in_maps):
            out_arrs = self.call(self.prep(in_maps), self.zeros())
            return [
                {
                    name: np.asarray(out_arrs[i]).reshape(
                        n_cores, *out_avals[i].shape
                    )[c]
                    for i, name in enumerate(out_names)
                }
                for c in range(n_cores)
            ]

    return Runner()


def host_prep_core(enc_core, dec_core, We, be, Wd, bd, Wf, cfg, consts=None):
    import ml_dtypes
    bf = ml_dtypes.bfloat16
    c = cfg
    tokens = np.ascontiguousarray(enc_core.reshape(c.TL, c.E))
    encN = tokens.astype(bf)
    encT = np.ascontiguousarray(encN.T.reshape(c.KO, 128, c.TL).transpose(1, 0, 2))
    if consts is None:
        we = np.ascontiguousarray(
            We.astype(bf).reshape(c.KO, 128, c.A).transpose(1, 0, 2))
        wd = np.ascontiguousarray(
            Wd.astype(bf).reshape(c.KD, 128, c.A).transpose(1, 0, 2))
        bea = np.ascontiguousarray((be + bd).astype(np.float32).reshape(c.AM, 128).T)
        wf = np.ascontiguousarray(Wf[:, 0].astype(bf).reshape(c.AM, 128).T)
        ident = np.eye(128, dtype=bf)
        consts = dict(we=we, wd=wd, bea=bea, wf=wf, ident=ident)
    dect = np.ascontiguousarray(
        dec_core.T.astype(bf).reshape(c.KD, 128, c.BL).transpose(1, 0, 2))
    return dict(encT=encT, encN=encN, dect=dect, **consts), consts


_CACHE = {}


def _get_runner():
    if "runner" not in _CACHE:
        cfg = Cfg()
        nc = build_program(cfg)
        _CACHE["cfg"] = cfg
        _CACHE["runner"] = build_runner(nc, N_CORES)
    return _CACHE["cfg"], _CACHE["runner"]


def kernel(encoder_out, decoder_hidden, We, be, Wd, bd, Wf, bf):
    """Full-input entry point. Shards batch over 8 cores, runs the Bass
    kernel SPMD, gathers full outputs. Returns (awe [B,E] f32, alpha [B,P]
    f32) matching the reference. (bf only shifts scores by a constant per
    row, which softmax cancels, so it does not enter the computation.)"""
    encoder_out = np.asarray(encoder_out, dtype=np.float32)
    decoder_hidden = np.asarray(decoder_hidden, dtype=np.float32)
    We, be = np.asarray(We, np.float32), np.asarray(be, np.float32)
    Wd, bd = np.asarray(Wd, np.float32), np.asarray(bd, np.float32)
    Wf = np.asarray(Wf, np.float32)

    cfg, runner = _get_runner()
    BL = cfg.BL
    consts = None
    in_maps = []
    for ci in range(N_CORES):
        m, consts = host_prep_core(
            encoder_out[ci * BL : (ci + 1) * BL],
            decoder_hidden[ci * BL : (ci + 1) * BL],
            We, be, Wd, bd, Wf, cfg, consts,
        )
        in_maps.append(m)
    res = runner.run(in_maps)
    awe = np.concatenate([res[i]["awe"] for i in range(N_CORES)], axis=0)
    alpha = np.concatenate([res[i]["alpha"] for i in range(N_CORES)], axis=0)
    return awe, alpha


# revision 5
# speedup vs baseline: 1.0522x; 1.0522x over previous
"""Bahdanau-attention kernel (nn_Attention_61357902791110) for Trainium2.

Data-parallel over 8 NeuronCores: the batch (128) is sharded 16 per core;
all weights are replicated. Inputs are cast/transposed host-side into the
layouts the device kernel wants; the Bass/Tile program per core computes

  att1 = enc @ We + be; att2 = dec @ Wd + bd
  s     = relu(att1 + att2[:,None,:]) @ Wf        (+bf dropped: softmax-inv.)
  alpha = softmax(s, axis=P)
  awe   = einsum('pe,p->e', enc, alpha)  per batch row

and returns (awe [128,2048] f32, alpha [128,196] f32).

Self-contained: only needs the toolchain at /opt/trn_rl_repo and 8 visible
neuron cores via jax.
"""

import sys

sys.path.insert(0, "/opt/trn_rl_repo")

from contextlib import ExitStack
from dataclasses import dataclass

import numpy as np

import concourse.bacc as bacc
import concourse.bass as bass  # noqa: F401  (engine types referenced via nc)
import concourse.tile as tile
from concourse import mybir

FP32 = mybir.dt.float32
BF16 = mybir.dt.bfloat16
AXX = mybir.AxisListType.X
AF = mybir.ActivationFunctionType
ALU = mybir.AluOpType

N_CORES = 8


@dataclass
class Cfg:
    BL: int = 16      # batches per core
    P: int = 196      # pixels
    E: int = 2048     # encoder dim
    A: int = 512      # attention dim
    D: int = 512      # decoder dim
    reps: int = 1     # in-kernel repetitions (timing harnesses only)

    @property
    def KO(self): return self.E // 128
    @property
    def AM(self): return self.A // 128
    @property
    def KD(self): return self.D // 128
    @property
    def NT(self): return 2 * self.P      # tokens per b-pair
    @property
    def NJ(self): return self.BL // 2
    @property
    def TL(self): return self.BL * self.P
    @property
    def NE(self): return self.E // 512   # awe psum chunks
    @property
    def P1(self): return min(self.P, 128)
    @property
    def P2(self): return self.P - self.P1


def emit_kernel(ctx: ExitStack, tc: tile.TileContext, ins: dict, outs: dict, cfg: Cfg):
    nc = tc.nc
    c = cfg
    encT_D, encN_D = ins["encT"], ins["encN"]
    we_D, wd_D, dect_D = ins["we"], ins["wd"], ins["dect"]
    bea_D, wf_D, id_D = ins["bea"], ins["wf"], ins["ident"]
    awe_D, alpha_D = outs["awe"], outs["alpha"]

    const = ctx.enter_context(tc.tile_pool(name="const", bufs=1))
    encT_pool = ctx.enter_context(tc.tile_pool(name="encT", bufs=2))
    encN_pool = ctx.enter_context(tc.tile_pool(name="encN", bufs=16))
    relu_pool = ctx.enter_context(tc.tile_pool(name="relu", bufs=2))
    small = ctx.enter_context(tc.tile_pool(name="small", bufs=3))
    pm = ctx.enter_context(tc.tile_pool(name="pm", bufs=1, space="PSUM"))
    pswf = ctx.enter_context(tc.tile_pool(name="pswf", bufs=1, space="PSUM"))
    pstr = ctx.enter_context(tc.tile_pool(name="pstr", bufs=1, space="PSUM"))
    psawe = ctx.enter_context(tc.tile_pool(name="psawe", bufs=2, space="PSUM"))

    NCH = 4
    KG = c.KO // NCH

    we_sb = const.tile([128, c.KO * c.A], BF16, tag="we")
    we_v = we_sb.rearrange("p (k a) -> p k a", k=c.KO)
    for g in range(NCH):
        nc.sync.dma_start(we_v[:, g * KG : (g + 1) * KG, :],
                          we_D[:, g * KG : (g + 1) * KG, :])

    def emit_dma_encT(j):
        t0 = j * c.NT
        encT = encT_pool.tile([128, c.KO * c.NT], BF16, tag="encT")
        encT_v = encT.rearrange("p (k t) -> p k t", k=c.KO)
        for g in range(NCH):
            nc.sync.dma_start(encT_v[:, g * KG : (g + 1) * KG, :],
                              encT_D[:, g * KG : (g + 1) * KG, t0 : t0 + c.NT])
        return encT_v

    SEG = c.P // 2          # K-segment for the awe matmuls
    NSEG = 8                # segments per quad (4 batches x 2)

    def emit_dma_encN(q):
        # 8 tiles of [SEG, E] covering the quad's 4*P tokens
        t0 = q * 4 * c.P
        encN_tiles = []
        for s in range(NSEG):
            ta = encN_pool.tile([SEG, c.E], BF16, tag="encN")
            nc.sync.dma_start(ta[:], encN_D[t0 + s * SEG : t0 + (s + 1) * SEG, :])
            encN_tiles.append(ta)
        return encN_tiles

    def emit_main_mm(j, encT_v):
        ps_main = [
            pm.tile([128, c.NT], FP32, tag=f"m{am}", name=f"psm{am}")
            for am in range(c.AM)
        ]
        for ko in range(c.KO):
            for am in range(c.AM):
                nc.tensor.matmul(
                    ps_main[am][:],
                    lhsT=we_sb[:, ko * c.A + am * 128 : ko * c.A + (am + 1) * 128],
                    rhs=encT_v[:, ko, :],
                    start=(ko == 0), stop=(ko == c.KO - 1),
                )
        return ps_main

    def emit_phase0():
        wd_sb = const.tile([128, c.KD * c.A], BF16, tag="wd")
        nc.sync.dma_start(wd_sb.rearrange("p (k a) -> p k a", k=c.KD), wd_D[:])
        dect_sb = const.tile([128, c.KD * c.BL], BF16, tag="dect")
        nc.sync.dma_start(dect_sb.rearrange("p (k b) -> p k b", k=c.KD), dect_D[:])
        bea_sb = const.tile([128, c.AM], FP32, tag="bea")
        nc.sync.dma_start(bea_sb[:], bea_D[:])
        wf_sb = const.tile([128, c.AM], BF16, tag="wf")
        nc.sync.dma_start(wf_sb[:], wf_D[:])
        id_sb = const.tile([128, 128], BF16, tag="ident")
        nc.sync.dma_start(id_sb[:], id_D[:])
        cT_sb = const.tile([128, c.AM * c.BL], FP32, tag="cT")
        for am in range(c.AM):
            ps = psawe.tile([128, c.BL], FP32, tag="a")
            for kd in range(c.KD):
                nc.tensor.matmul(
                    ps[:],
                    lhsT=wd_sb[:, kd * c.A + am * 128 : kd * c.A + (am + 1) * 128],
                    rhs=dect_sb[:, kd * c.BL : (kd + 1) * c.BL],
                    start=(kd == 0), stop=(kd == c.KD - 1),
                )
            nc.vector.tensor_scalar_add(
                cT_sb[:, am * c.BL : (am + 1) * c.BL], ps[:],
                bea_sb[:, am : am + 1],
            )
        return wf_sb, id_sb, cT_sb

    def emit_score_pre(j, ps_main, wf_sb, cT_sb, sc_q):
        """relu + Wf reduction + move scores into quad tile rows."""
        relu = relu_pool.tile([128, c.AM * c.NT], BF16, tag="relu")
        for am in range(c.AM):
            for h in range(2):
                b = 2 * j + h
                nc.vector.tensor_scalar(
                    relu[:, am * c.NT + h * c.P : am * c.NT + (h + 1) * c.P],
                    ps_main[am][:, h * c.P : (h + 1) * c.P],
                    cT_sb[:, am * c.BL + b : am * c.BL + b + 1],
                    0.0,
                    op0=ALU.add, op1=ALU.max,
                )
        ps_s = pswf.tile([1, c.NT], FP32, tag="s")
        for am in range(c.AM):
            nc.tensor.matmul(
                ps_s[:],
                lhsT=wf_sb[:, am : am + 1],
                rhs=relu[:, am * c.NT : (am + 1) * c.NT],
                start=(am == 0), stop=(am == c.AM - 1),
            )
        scr = small.tile([1, c.NT], FP32, tag="scr")
        nc.scalar.copy(scr[:], ps_s[:])
        r0 = 2 * (j % 2)
        for h in range(2):
            nc.sync.dma_start(sc_q[r0 + h : r0 + h + 1, :],
                              scr[0:1, h * c.P : (h + 1) * c.P])

    def emit_softmax_q(q, sc_q, id_sb):
        """Quad softmax [4, P], alpha out, and the block-diagonal alphaT
        [SEG, NSEG*4] used as stationary operands by the awe matmuls."""
        nmax = small.tile([4, 1], FP32, tag="nmax")
        nc.vector.reduce_max(nmax[:], sc_q[:], axis=AXX, negate=True)
        esb = small.tile([4, c.P], FP32, tag="esb")
        ssum = small.tile([4, 1], FP32, tag="ssum")
        nc.scalar.activation(esb[:], sc_q[:], AF.Exp, bias=nmax[:], accum_out=ssum[:])
        rsum = small.tile([4, 1], FP32, tag="rsum")
        nc.vector.reciprocal(rsum[:], ssum[:])
        al_f = small.tile([4, c.P], FP32, tag="al_f")
        nc.vector.tensor_scalar_mul(al_f[:], esb[:], rsum[:])
        al_b = small.tile([4, c.P], BF16, tag="al_b")
        nc.vector.tensor_copy(al_b[:], al_f[:])
        nc.sync.dma_start(alpha_D[4 * q : 4 * q + 4, :], al_f[:])
        pt = pstr.tile([SEG, 8], BF16, tag="t")
        nc.tensor.transpose(pt[:, 0:4], al_b[:, 0:SEG], id_sb[0:4, 0:4])
        nc.tensor.transpose(pt[:, 4:8], al_b[:, SEG : c.P], id_sb[0:4, 0:4])
        alphaT = small.tile([SEG, NSEG * 4], BF16, tag="alphaT")
        nc.gpsimd.memset(alphaT[:], 0.0)
        for s in range(NSEG):
            b_idx = s // 2
            half = s % 2
            nc.vector.tensor_copy(
                alphaT[:, s * 4 + b_idx : s * 4 + b_idx + 1],
                pt[:, half * 4 + b_idx : half * 4 + b_idx + 1],
            )
        return alphaT

    def emit_awe_q(q, alphaT, encN_tiles, awe_sb, ns):
        for n in ns:
            pa = psawe.tile([4, 512], FP32, tag="a")
            for s in range(NSEG):
                nc.tensor.matmul(
                    pa[:],
                    lhsT=alphaT[:, s * 4 : (s + 1) * 4],
                    rhs=encN_tiles[s][:, n * 512 : (n + 1) * 512],
                    start=(s == 0), stop=(s == NSEG - 1),
                )
            nc.scalar.copy(awe_sb[:, n * 512 : (n + 1) * 512], pa[:])
        if ns[-1] == c.NE - 1:
            nc.sync.dma_start(awe_D[4 * q : 4 * q + 4, :], awe_sb[:])

    def body():
        consts = None
        prev = None   # (q, alphaT, encN_tiles)
        sc_q = None
        pend = None
        for j in range(c.NJ):
            q, jh = divmod(j, 2)
            encT_v = emit_dma_encT(j)
            ps_main = emit_main_mm(j, encT_v)
            if j == 0:
                consts = emit_phase0()
            wf_sb, id_sb, cT_sb = consts
            if jh == 0:
                sc_q = small.tile([4, c.P], FP32, tag="sc", name="sc_q")
                pend = emit_dma_encN(q)
            if prev is not None:
                half = c.NE // 2
                ns = list(range(0, half)) if jh == 0 else list(range(half, c.NE))
                emit_awe_q(prev[0], prev[1], prev[2], prev[3], ns)
                if jh == 1:
                    prev = None
            emit_score_pre(j, ps_main, wf_sb, cT_sb, sc_q)
            if jh == 1:
                alphaT = emit_softmax_q(q, sc_q, id_sb)
                awe_sb = small.tile([4, c.E], FP32, tag="awe_sb", name="awe_sb")
                prev = (q, alphaT, pend, awe_sb)
        emit_awe_q(prev[0], prev[1], prev[2], prev[3], list(range(c.NE)))

    if c.reps == 1:
        body()
    else:
        with tc.For_i(0, c.reps, 1,
                      hint_engines=(mybir.EngineType.PE,)):
            body()


def build_program(cfg: Cfg):
    c = cfg
    nc = bacc.Bacc("TRN2", target_bir_lowering=False, debug=False)
    ins = {}
    outs = {}
    ins["encT"] = nc.dram_tensor("encT", [128, c.KO, c.TL], BF16, kind="ExternalInput").ap()
    ins["encN"] = nc.dram_tensor("encN", [c.TL, c.E], BF16, kind="ExternalInput").ap()
    ins["we"] = nc.dram_tensor("we", [128, c.KO, c.A], BF16, kind="ExternalInput").ap()
    ins["wd"] = nc.dram_tensor("wd", [128, c.KD, c.A], BF16, kind="ExternalInput").ap()
    ins["dect"] = nc.dram_tensor("dect", [128, c.KD, c.BL], BF16, kind="ExternalInput").ap()
    ins["bea"] = nc.dram_tensor("bea", [128, c.AM], FP32, kind="ExternalInput").ap()
    ins["wf"] = nc.dram_tensor("wf", [128, c.AM], BF16, kind="ExternalInput").ap()
    ins["ident"] = nc.dram_tensor("ident", [128, 128], BF16, kind="ExternalInput").ap()
    outs["awe"] = nc.dram_tensor("awe", [c.BL, c.E], FP32, kind="ExternalOutput").ap()
    outs["alpha"] = nc.dram_tensor("alpha", [c.BL, c.P], FP32, kind="ExternalOutput").ap()

    with tile.TileContext(nc) as tc:
        with ExitStack() as ctx:
            emit_kernel(ctx, tc, ins, outs, cfg)
    nc.compile()
    return nc


def build_runner(nc, n_cores):
    """Persistent SPMD runner (port of bass2jax.run_bass_via_pjrt that keeps
    the jitted executable so repeated calls don't re-trace)."""
    import jax
    import jax.core
    from jax.experimental.shard_map import shard_map
    from jax.sharding import Mesh, PartitionSpec

    from concourse.bass2jax import (
        _bass_exec_p,
        install_neuronx_cc_hook,
        partition_id_tensor,
    )

    install_neuronx_cc_hook()
    partition_name = nc.partition_id_tensor.name if nc.partition_id_tensor else None
    in_names, out_names, out_avals, zero_outs = [], [], [], []
    for alloc in nc.m.functions[0].allocations:
        if not isinstance(alloc, mybir.MemoryLocationSet):
            continue
        name = alloc.memorylocations[0].name
        if alloc.kind == "ExternalInput":
            if name != partition_name:
                in_names.append(name)
        elif alloc.kind == "ExternalOutput":
            out_names.append(name)
            shape = tuple(alloc.tensor_shape)
            dtype = mybir.dt.np(alloc.dtype)
            out_avals.append(jax.core.ShapedArray(shape, dtype))
            zero_outs.append(np.zeros(shape, dtype))
    n_params = len(in_names)
    n_outs = len(out_avals)
    all_in = in_names + out_names + ([partition_name] if partition_name else [])

    def _body(*args):
        operands = list(args)
        if partition_name is not None:
            operands.append(partition_id_tensor())
        outs = _bass_exec_p.bind(
            *operands,
            out_avals=tuple(out_avals),
            in_names=tuple(all_in),
            out_names=tuple(out_names),
            lowering_input_output_aliases=(),
            sim_require_finite=True,
            sim_require_nnan=True,
            nc=nc,
        )
        return tuple(outs)

    devices = jax.devices()[:n_cores]
    mesh = Mesh(np.asarray(devices), ("core",))
    donate = tuple(range(n_params, n_params + n_outs))
    sharded = jax.jit(
        shard_map(
            _body,
            mesh=mesh,
            in_specs=(PartitionSpec("core"),) * (n_params + n_outs),
            out_specs=(PartitionSpec("core"),) * n_outs,
            check_rep=False,
        ),
        donate_argnums=donate,
        keep_unused=True,
    )

    class Runner:
        def __init__(self):
            self.in_names = in_names
            self.out_names = out_names
            self.out_avals = out_avals
            self.mesh = mesh
            self.n_cores = n_cores

        def prep(self, in_maps):
            return [
                np.concatenate([np.asarray(m[name]) for m in in_maps], axis=0)
                for name in in_names
            ]

        def zeros(self):
            return [
                np.zeros((n_cores * z.shape[0], *z.shape[1:]), z.dtype)
                for z in zero_outs
            ]

        def call(self, concat_in, concat_zeros):
            return sharded(*concat_in, *concat_zeros)

        def run(self, in_maps):
            out_arrs = self.call(self.prep(in_maps), self.zeros())
            return [
                {
                    name: np.asarray(out_arrs[i]).reshape(
                        n_cores, *out_avals[i].shape
                    )[c]
                    for i, name in enumerate(out_names)
                }
                for c in range(n_cores)
            ]

    return Runner()


def host_prep_core(enc_core, dec_core, We, be, Wd, bd, Wf, cfg, consts=None):
    import ml_dtypes
    bf = ml_dtypes.bfloat16
    c = cfg
    tokens = np.ascontiguousarray(enc_core.reshape(c.TL, c.E))
    encN = tokens.astype(bf)
    encT = np.ascontiguousarray(encN.T.reshape(c.KO, 128, c.TL).transpose(1, 0, 2))
    if consts is None:
        we = np.ascontiguousarray(
            We.astype(bf).reshape(c.KO, 128, c.A).transpose(1, 0, 2))
        wd = np.ascontiguousarray(
            Wd.astype(bf).reshape(c.KD, 128, c.A).transpose(1, 0, 2))
        bea = np.ascontiguousarray((be + bd).astype(np.float32).reshape(c.AM, 128).T)
        wf = np.ascontiguousarray(Wf[:, 0].astype(bf).reshape(c.AM, 128).T)
        ident = np.eye(128, dtype=bf)
        consts = dict(we=we, wd=wd, bea=bea, wf=wf, ident=ident)
    dect = np.ascontiguousarray(
        dec_core.T.astype(bf).reshape(c.KD, 128, c.BL).transpose(1, 0, 2))
    return dict(encT=encT, encN=encN, dect=dect, **consts), consts


_CACHE = {}


def _get_runner():
    if "runner" not in _CACHE:
        cfg = Cfg()
        nc = build_program(cfg)
        _CACHE["cfg"] = cfg
        _CACHE["runner"] = build_runner(nc, N_CORES)
    return _CACHE["cfg"], _CACHE["runner"]


def kernel(encoder_out, decoder_hidden, We, be, Wd, bd, Wf, bf):
    """Full-input entry point. Shards batch over 8 cores, runs the Bass
    kernel SPMD, gathers full outputs. Returns (awe [B,E] f32, alpha [B,P]
    f32) matching the reference. (bf only shifts scores by a constant per
    row, which softmax cancels, so it does not enter the computation.)"""
    encoder_out = np.asarray(encoder_out, dtype=np.float32)
    decoder_hidden = np.asarray(decoder_hidden, dtype=np.float32)
    We, be = np.asarray(We, np.float32), np.asarray(be, np.float32)
    Wd, bd = np.asarray(Wd, np.float32), np.asarray(bd, np.float32)
    Wf = np.asarray(Wf, np.float32)

    cfg, runner = _get_runner()
    BL = cfg.BL
    consts = None
    in_maps = []
    for ci in range(N_CORES):
        m, consts = host_prep_core(
            encoder_out[ci * BL : (ci + 1) * BL],
            decoder_hidden[ci * BL : (ci + 1) * BL],
            We, be, Wd, bd, Wf, cfg, consts,
        )
        in_maps.append(m)
    res = runner.run(in_maps)
    awe = np.concatenate([res[i]["awe"] for i in range(N_CORES)], axis=0)
    alpha = np.concatenate([res[i]["alpha"] for i in range(N_CORES)], axis=0)
    return awe, alpha
